# revision 1
# baseline (speedup 1.0000x reference)
"""AdaptiveFusion (gated fusion + LayerNorm) distributed Trainium2 kernel, v2.

Math (per token, D=1024):
  logit_c = x1 . W1[c] + x2 . W2[c]           (c = 0, 1)
  lam_c   = sigmoid(logit_c)
  fused   = (1+lam_1)*x1 + (1+lam_2)*x2
  out     = LayerNorm(fused)                  (eps=1e-5; gamma/beta host-side)

Sharding: data-parallel over tokens, 8 shards of 4096 tokens.

v2 engine plan (per 128-token subtile, bf16 I/O):
 - PE: 16x transpose of the x tile into PSUM (bf16), then 16x matmul
   with the transposed chunks as stationary and the gate weights
   [128,2] as moving, accumulating logits [128 tok, 2] in PSUM. This
   removes both big mul+reduce gate passes from DVE/ACT.
 - DVE: tensor_scalar copy of the transposed x PSUM->SBUF (the matmul
   stationary must live in SBUF), the fused custom op (sum accum gives
   the LN mean), and the small sigmoid/variance chains.
 - ACT: Exp for sigmoid, Square+accum for E[fused^2], Ln+Exp for rstd,
   Identity(scale=rstd, bias=-mean*rstd) epilogue. One table set.
"""

import numpy as np
import ml_dtypes

import concourse.bacc as bacc
import concourse.bass as bass
import concourse.mybir as mybir
from concourse.bass_utils import run_bass_kernel_spmd
from concourse.tile import TileContext

BF16 = mybir.dt.bfloat16
F32 = mybir.dt.float32


def _make_fused_sum_op():
    """out = in0*(s0+1) + in1*(s1+1); accum_out = sum(out). Same op as the
    v1 kernel (FUSED_SUM1_ANT), self-pinning uops sha at first compile."""
    import re
    from operator import add

    import concourse.dve_ops as dve_ops
    from concourse.dve_spec import Spec, Src0, Src1, C0, C1, Zero

    def _ref(in0, in1, s0, s1, imm2):
        b = (
            in0.astype(np.float32) * (s0 + 1.0)
            + in1.astype(np.float32) * (s1 + 1.0)
        ).astype(np.float32)
        return b, b.reshape(b.shape[0], -1).sum(axis=-1, keepdims=True)

    for existing in dve_ops.OPS:
        if existing.name == "FUSED_SUM1_ANT":
            return existing

    spec = Spec(
        body=(Src0 * C0 + Src1 * C1) + (Src0 + Src1),
        accum=add, accum_init=Zero, reference=_ref,
    )
    op = dve_ops.DveOp("FUSED_SUM1_ANT", spec, subdim=False, uops_sha={})
    dve_ops.OPS.append(op)
    dve_ops._SUB_OPCODE_FOR_NAME[op.name] = (
        dve_ops._CUSTOM_DVE_ROW_BASE + len(dve_ops.OPS) - 1
    )
    dve_ops.CUSTOM_DVE_SPECS[op.name] = spec
    assert dve_ops._SUB_OPCODE_FOR_NAME[op.name] < 0x20
    for ver in ("v3", "v4"):
        try:
            op.compile(ver)
        except ValueError as e:
            m = re.search(r'="([0-9a-f]{16})"', str(e))
            if not m:
                raise
            op.uops_sha[ver] = m.group(1)
            dve_ops._COMPILE_CACHE.pop((op.name, ver), None)
            op.compile(ver)
    return op


FUSED_SUM = _make_fused_sum_op()


def _pin_act_table_set():
    from concourse.hw_specs import get_activation_tables

    AF = mybir.ActivationFunctionType
    mine = {AF.Exp, AF.Ln, AF.Copy, AF.Square, AF.Identity, AF.MemsetZero}
    tabs = get_activation_tables("gen3")
    assert mine <= tabs["natural_log_exp_and_others"]
    for name, s in tabs.items():
        if name != "natural_log_exp_and_others":
            s -= mine


B, T, D = 8, 4096, 1024
N_CORES = 8
N_TOK = B * T
TOK_PER_CORE = N_TOK // N_CORES  # 4096
P = 128
SUB = 4
GROUP = P * SUB                  # 512 tokens per group
N_GROUPS = TOK_PER_CORE // GROUP # 8
LN_EPS = 1e-5
NCHUNK = 16                      # 2048 / 128 d-chunks

# balance knob: fraction of the xT PSUM->SBUF copy done by ACT (rest DVE)
ACT_COPY_COLS = 0

_CACHE = {}


def _build():
    _pin_act_table_set()
    nc = bacc.Bacc()
    x = nc.declare_dram_parameter("x", [TOK_PER_CORE, 2 * D], BF16, isOutput=False)
    wc = nc.declare_dram_parameter("wc", [P, NCHUNK, 2], BF16, isOutput=False)
    ident = nc.declare_dram_parameter("ident", [P, P], BF16, isOutput=False)
    out = nc.declare_dram_parameter("out", [TOK_PER_CORE, D], BF16, isOutput=True)

    mult = mybir.AluOpType.mult
    addop = mybir.AluOpType.add
    AF = mybir.ActivationFunctionType

    with TileContext(nc) as tc:
        with (
            tc.tile_pool(name="wpool", bufs=1) as wpool,
            tc.tile_pool(name="xpool", bufs=4) as xpool,
            tc.tile_pool(name="xtpool", bufs=4) as xtpool,
            tc.tile_pool(name="fpool", bufs=8) as fpool,
            tc.tile_pool(name="opool", bufs=3) as opool,
            tc.tile_pool(name="small", bufs=4) as spool,
            tc.tile_pool(name="psT", bufs=5, space="PSUM") as psTp,
            tc.tile_pool(name="psG", bufs=3, space="PSUM") as psGp,
        ):
            wt = wpool.tile([P, NCHUNK, 2], BF16)
            idt = wpool.tile([P, P], BF16)
            nc.sync.dma_start(out=wt[:], in_=wc[:, :, :])
            nc.sync.dma_start(out=idt[:], in_=ident[:, :])

            state = {}

            def emit_group_in(g):
                xt = xpool.tile([P, SUB, 2 * D], BF16, tag="xt", name="xtt")
                xre = x[g * GROUP : (g + 1) * GROUP, :].rearrange(
                    "(j p) c -> p j c", p=P)
                if g == 0:
                    for j in range(SUB):
                        nc.sync.dma_start(out=xt[:, j, :], in_=xre[:, j, :])
                else:
                    nc.sync.dma_start(out=xt[:], in_=xre)
                state[("xt", g)] = xt
                state[("gp", g)] = psGp.tile([P, SUB, 2], F32, tag="gp", name="gpt")

            def emit_transposes(si):
                # two half-subtile PSUM tiles (1 bank each) so the DVE copy
                # of the low half overlaps the high half's transposes
                g, j = divmod(si, SUB)
                xt = state[("xt", g)]
                halves = []
                for h in range(2):
                    pT = psTp.tile([P, D], BF16, tag="pT", name="pTt")
                    for k in range(NCHUNK // 2):
                        kk = h * (NCHUNK // 2) + k
                        nc.tensor.transpose(
                            pT[:, k * P : (k + 1) * P],
                            xt[:, j, kk * P : (kk + 1) * P],
                            idt[:],
                        )
                    halves.append(pT)
                state[("pT", si)] = halves

            def emit_copy(si):
                halves = state.pop(("pT", si))
                xts = xtpool.tile([P, 2 * D], BF16, tag="xts", name="xtst")
                # ALL copies on DVE: they feed the PE (matmul stationary);
                # routing any through ACT's deep FIFO stalls the PE chain
                # (measured: +23us)
                for h in range(2):
                    nc.vector.tensor_scalar(
                        out=xts[:, h * D : (h + 1) * D], in0=halves[h][:],
                        scalar1=1.0, scalar2=None, op0=mult)
                state[("xts", si)] = xts

            def emit_matmuls(si):
                g, j = divmod(si, SUB)
                xts = state.pop(("xts", si))
                gp = state[("gp", g)]
                for k in range(NCHUNK):
                    nc.tensor.matmul(
                        gp[:, j, :],
                        lhsT=xts[:, k * P : (k + 1) * P],
                        rhs=wt[:, k, :],
                        start=(k == 0),
                        stop=(k == NCHUNK - 1),
                    )

            def emit_tail_a(g):
                heavy = g >= N_GROUPS - 2
                gp = state.pop(("gp", g))
                xt = state.pop(("xt", g))
                # lam = 1 / (1 + exp(-l)) ; batched [P, SUB, 2]
                e8 = spool.tile([P, SUB, 2], F32, tag="e8", name="e8t")
                nc.scalar.activation(e8[:], gp[:], AF.Exp, scale=-1.0)
                p8 = spool.tile([P, SUB, 2], F32, tag="p8", name="p8t")
                nc.vector.tensor_scalar_add(p8[:], e8[:], 1.0)
                r8 = spool.tile([P, SUB, 2], F32, tag="r8", name="r8t")
                nc.vector.reciprocal(r8[:], p8[:])

                s4 = spool.tile([P, SUB], F32, tag="s4", name="s4t")
                q4 = spool.tile([P, SUB], F32, tag="q4", name="q4t")
                fused = [None] * SUB
                for j in range(SUB):
                    fused[j] = fpool.tile([P, D], BF16, tag=f"fu{j}", name=f"fut{j}")
                    nc.vector._custom_dve(
                        FUSED_SUM,
                        out=fused[j][:],
                        in0=xt[:, j, 0:D],
                        in1=xt[:, j, D : 2 * D],
                        s0=r8[:, j, 0:1],
                        s1=r8[:, j, 1:2],
                        accum_out=s4[:, j : j + 1],
                    )
                    sqj = fpool.tile([P, D], BF16, tag="sqj", name="sqjt")
                    nc.scalar.activation(
                        sqj[:], fused[j][:], AF.Square,
                        accum_out=q4[:, j : j + 1])
                state[("tail", g)] = (s4, q4, fused)

            def emit_tail_b(g):
                heavy = g >= N_GROUPS - 2
                s4, q4, fused = state.pop(("tail", g))
                # var = q/D - (s/D)^2 ; rstd = exp(-0.5*ln(var+eps))
                # e2 = q/D + eps (eps folded so Ln needs no bias const)
                e2 = spool.tile([P, SUB], F32, tag="e2", name="e2t")
                nc.vector.tensor_scalar(
                    out=e2[:], in0=q4[:], scalar1=1.0 / D, scalar2=LN_EPS,
                    op0=mult, op1=addop)
                m2 = spool.tile([P, SUB], F32, tag="m2", name="m2t")
                nc.vector.tensor_mul(m2[:], s4[:], s4[:])
                var4 = spool.tile([P, SUB], F32, tag="var4", name="var4t")
                nc.vector.scalar_tensor_tensor(
                    out=var4[:], in0=m2[:], scalar=-1.0 / (D * D), in1=e2[:],
                    op0=mult, op1=addop)
                L4 = spool.tile([P, SUB], F32, tag="L4", name="L4t")
                nc.scalar.activation(L4[:], var4[:], AF.Ln)
                rstd4 = spool.tile([P, SUB], F32, tag="rstd4", name="rstd4t")
                nc.scalar.activation(rstd4[:], L4[:], AF.Exp, scale=-0.5)
                nb4 = spool.tile([P, SUB], F32, tag="nb4", name="nb4t")
                nc.vector.scalar_tensor_tensor(
                    out=nb4[:], in0=s4[:], scalar=-1.0 / D, in1=rstd4[:],
                    op0=mult, op1=mult)

                if heavy:
                    mean4 = spool.tile([P, SUB], F32, tag="mean4", name="mean4t")
                    nc.vector.tensor_scalar_mul(mean4[:], s4[:], 1.0 / D)
                ot = opool.tile([P, SUB, D], BF16, tag="ot", name="ott")
                for j in range(SUB):
                    if heavy:
                        nc.vector.tensor_scalar(
                            out=ot[:, j, :], in0=fused[j][:],
                            scalar1=mean4[:, j : j + 1],
                            scalar2=rstd4[:, j : j + 1],
                            op0=mybir.AluOpType.subtract, op1=mult,
                        )
                    else:
                        nc.scalar.activation(
                            ot[:, j, :], fused[j][:], AF.Identity,
                            bias=nb4[:, j : j + 1], scale=rstd4[:, j : j + 1],
                        )
                nc.sync.dma_start(
                    out=out[g * GROUP : (g + 1) * GROUP, :].rearrange(
                        "(j p) c -> p j c", p=P),
                    in_=ot[:],
                )

            # ---- emission schedule: PE stages pipelined one subtile deep;
            # group tails split in two half-group stages two groups behind
            nsub = N_GROUPS * SUB
            emit_group_in(0)
            emit_transposes(0)
            for si in range(1, nsub):
                g, j = divmod(si, SUB)
                if j == 0:
                    emit_group_in(g)
                emit_copy(si - 1)
                emit_transposes(si)
                emit_matmuls(si - 1)
                if j == 0 and g >= 2:
                    emit_tail_a(g - 2)
                if j == 2 and g >= 2:
                    emit_tail_b(g - 2)
            emit_copy(nsub - 1)
            emit_matmuls(nsub - 1)
            emit_tail_a(N_GROUPS - 2)
            emit_tail_b(N_GROUPS - 2)
            emit_tail_a(N_GROUPS - 1)
            emit_tail_b(N_GROUPS - 1)
    nc.finalize()
    return nc


def _get_nc():
    if "nc" not in _CACHE:
        _CACHE["nc"] = _build()
    return _CACHE["nc"]


def _host_inputs(input_1, input_2, W1, W2):
    bf16 = ml_dtypes.bfloat16
    x1 = np.ascontiguousarray(np.asarray(input_1, dtype=np.float32).reshape(N_TOK, D))
    x2 = np.ascontiguousarray(np.asarray(input_2, dtype=np.float32).reshape(N_TOK, D))
    xcat = np.empty((N_TOK, 2 * D), dtype=bf16)
    xcat[:, :D] = x1
    xcat[:, D:] = x2
    W1 = np.asarray(W1, dtype=np.float32)
    W2 = np.asarray(W2, dtype=np.float32)
    wc = np.zeros((P, NCHUNK, 2), dtype=np.float32)
    for k in range(8):
        wc[:, k, 0] = W1[0, k * P : (k + 1) * P]
        wc[:, k, 1] = W1[1, k * P : (k + 1) * P]
    for k in range(8, NCHUNK):
        wc[:, k, 0] = W2[0, (k - 8) * P : (k - 7) * P]
        wc[:, k, 1] = W2[1, (k - 8) * P : (k - 7) * P]
    ident = np.eye(P, dtype=np.float32)
    return xcat, wc.astype(bf16), ident.astype(bf16)


def kernel(input_1, input_2, W1, W2, ln_gamma, ln_beta, _trace=False):
    xcat, wc, ident = _host_inputs(input_1, input_2, W1, W2)
    nc = _get_nc()
    in_maps = [
        {
            "x": xcat[i * TOK_PER_CORE : (i + 1) * TOK_PER_CORE],
            "wc": wc,
            "ident": ident,
        }
        for i in range(N_CORES)
    ]
    res = run_bass_kernel_spmd(
        nc, in_maps, core_ids=list(range(N_CORES)), trace=_trace
    )
    out = np.concatenate(
        [res.results[i]["out"].astype(np.float32) for i in range(N_CORES)], axis=0
    )
    out = out.reshape(B, T, D)
    g = np.asarray(ln_gamma, dtype=np.float32)
    b = np.asarray(ln_beta, dtype=np.float32)
    if not (np.all(g == 1.0) and np.all(b == 0.0)):
        out = out * g + b
    if _trace:
        return out, res
    return out

